# revision 7
# baseline (speedup 1.0000x reference)
"""Trainium2 Bass kernel for 2-layer LSTM (T=512, B=64, IN=H=512).

Strategy (data-parallel over batch, per sharding hint):
  - 8 cores, each handles B_loc = 8 batch elements with replicated weights.
  - All tensors on-chip use a transposed layout: partitions = gate/hidden
    dims, free dim = (time|kc) x batch. Gates live as [128, 16mc x 8b].
  - Input projections (W_ih @ x + b) are batched over blocks of 32 steps
    (moving operand N=256) and interleaved with the recurrent loop.
  - Recurrent step: 64 weight-stationary matmuls (bf16, FWL) accumulate
    W_hh.T chunks against h_{t-1} (N=8), + Z_t add (DVE), tanh-trick
    activations (ACT), fused scalar_tensor_tensor cell update (DVE).
  - tanh trick: sigma(x) = (1+tanh(x/2))/2; g-rows of W/b pre-scaled by 2
    so ONE tanh op covers all 4 gates. States stored scaled: h''=2h, c''=2c
    (W_hh, W_ih1 pre-scaled by 0.5 to compensate; outputs halved on host).
"""

import os
import sys

import numpy as np

for p in ("/opt/trn_rl_repo", "/root/.axon_site/_ro/trn_rl_repo"):
    if os.path.isdir(p) and p not in sys.path:
        sys.path.insert(0, p)

import ml_dtypes  # noqa: E402

BF16 = ml_dtypes.bfloat16

T, B, IN, H, L = 512, 64, 512, 512, 2
NCORES = 8
BL = B // NCORES          # 8 batch per core
GD = 4 * H                # 2048 gate dims
MC = GD // 128            # 16 gate chunks
KC = H // 128             # 4 contraction chunks
TBLK = 32                 # steps per projection block
NBLK = T // TBLK          # 16 blocks
HS = 4672                 # hist per-kc col stride (>= 16*256+8+256, 32B align)
XS = 4608                 # padded xt free dim (cols), >= 17*256

_NC_CACHE = {}


def _build_nc():
    import concourse.bacc as bacc
    import concourse.bass as bass
    import concourse.mybir as mybir
    from concourse.tile import TileContext

    f32 = mybir.dt.float32
    bf16 = mybir.dt.bfloat16
    Tanh = mybir.ActivationFunctionType.Tanh
    Ident = mybir.ActivationFunctionType.Identity
    add = mybir.AluOpType.add
    mult = mybir.AluOpType.mult
    ds = bass.ds

    nc = bacc.Bacc("TRN2", target_bir_lowering=False)

    xt = nc.dram_tensor("xt", [128, KC, XS], bf16, kind="ExternalInput")
    w_ih0 = nc.dram_tensor("w_ih0", [128, KC * GD], bf16, kind="ExternalInput")
    w_hh0 = nc.dram_tensor("w_hh0", [128, KC * GD], bf16, kind="ExternalInput")
    w_ih1 = nc.dram_tensor("w_ih1", [128, KC * GD], bf16, kind="ExternalInput")
    w_hh1 = nc.dram_tensor("w_hh1", [128, KC * GD], bf16, kind="ExternalInput")
    biases = nc.dram_tensor("biases", [128, 2 * MC], f32, kind="ExternalInput")
    outb = nc.dram_tensor("outb", [T * 128, 4 * BL], f32, kind="ExternalOutput")
    fin = nc.dram_tensor("fin", [4 * 128, 4 * BL], f32, kind="ExternalOutput")

    with TileContext(nc) as tc:
        with (
            tc.tile_pool(name="wp", bufs=1) as wp,
            tc.tile_pool(name="dp", bufs=1) as dp,
            tc.tile_pool(name="tp", bufs=3) as tp,
            tc.tile_pool(name="pp", bufs=4, space="PSUM") as pp,
            tc.tile_pool(name="gp", bufs=2, space="PSUM") as gp,
        ):
            wt = {}
            for name, dram in (("ih0", w_ih0), ("hh0", w_hh0),
                               ("ih1", w_ih1), ("hh1", w_hh1)):
                wtile = wp.tile([128, KC * GD], bf16, name=f"w_{name}", tag=f"w_{name}")
                nc.sync.dma_start(out=wtile[:], in_=dram[:])
                wt[name] = wtile
            biast = wp.tile([128, 2 * MC], f32, tag="biast")
            nc.sync.dma_start(out=biast[:], in_=biases[:])

            hist = dp.tile([128, KC * HS], bf16, tag="hist")
            hist3 = hist[:].rearrange("p (k c) -> p k c", k=KC)
            xb = [dp.tile([128, KC * TBLK * BL], bf16, name=f"xb{e}", tag=f"xb{e}")
                  for e in range(2)]
            zt = [dp.tile([128, TBLK * 128], f32, name=f"z{e}", tag=f"z{e}") for e in range(2)]
            # persistent states (ping-pong)
            c0 = [dp.tile([128, KC * BL], f32, name=f"c0_{e}", tag=f"c0_{e}") for e in range(2)]
            h0 = [dp.tile([128, KC * BL], f32, name=f"h0_{e}", tag=f"h0_{e}") for e in range(2)]
            c1 = [dp.tile([128, KC * BL], f32, name=f"c1_{e}", tag=f"c1_{e}") for e in range(2)]
            h1 = [dp.tile([128, KC * BL], f32, name=f"h1_{e}", tag=f"h1_{e}") for e in range(2)]
            h0b = [dp.tile([128, KC * BL], bf16, name=f"h0b_{e}", tag=f"h0b_{e}") for e in range(2)]
            h1b = [dp.tile([128, KC * BL], bf16, name=f"h1b_{e}", tag=f"h1b_{e}") for e in range(2)]

            nc.vector.memset(hist[:], 0)
            for t_ in (c0[0], c1[0]):
                nc.vector.memset(t_[:], 0)
            for t_ in (h0b[0], h1b[0]):
                nc.vector.memset(t_[:], 0)

            def proj(wname, rhs_fn, zdst, bias_col0):
                """Z[s*128 + mc*8 + b] = sum_kc W.T chunk @ rhs + bias."""
                z3 = zdst[:].rearrange("p (s g) -> p s g", s=TBLK)
                w = wt[wname]
                for mc in range(MC):
                    pt = pp.tile([128, TBLK * BL], f32, tag="pt")
                    for kc in range(KC):
                        nc.tensor.matmul(
                            pt[:],
                            lhsT=w[:, kc * GD + mc * 128: kc * GD + mc * 128 + 128],
                            rhs=rhs_fn(kc),
                            start=(kc == 0), stop=(kc == KC - 1))
                    pt3 = pt[:].rearrange("p (s b) -> p s b", s=TBLK)
                    nc.scalar.activation(
                        z3[:, :, mc * BL: (mc + 1) * BL], pt3, Ident,
                        bias=biast[:, bias_col0 + mc: bias_col0 + mc + 1],
                        scale=1.0)

            def step(layer, iv, j, zsrc):
                """One recurrent step. t = iv*64 + j."""
                whh = wt["hh1"] if layer else wt["hh0"]
                hbf = h1b if layer else h0b
                cst = c1 if layer else c0
                hst = h1 if layer else h0
                rd, wr = j % 2, (j + 1) % 2

                g = gp.tile([128, MC * BL], f32, tag="g")
                for mc in range(MC):
                    for kc in range(KC):
                        nc.tensor.matmul(
                            g[:, mc * BL: (mc + 1) * BL],
                            lhsT=whh[:, kc * GD + mc * 128: kc * GD + mc * 128 + 128],
                            rhs=hbf[rd][:, kc * BL: (kc + 1) * BL],
                            start=(kc == 0), stop=(kc == KC - 1))
                gs = tp.tile([128, MC * BL], f32, tag="gs")
                nc.vector.tensor_add(gs[:], zsrc[:, j % TBLK * 128: (j % TBLK + 1) * 128], g[:])
                tt = tp.tile([128, MC * BL], f32, tag="tt")
                nc.scalar.activation(tt[:], gs[:], Tanh, scale=0.5)
                ti, tf = tt[:, 0:32], tt[:, 32:64]
                to, tg = tt[:, 64:96], tt[:, 96:128]
                v = tp.tile([128, KC * BL], f32, tag="v")
                nc.vector.scalar_tensor_tensor(v[:], ti, 1.0, tg, op0=add, op1=mult)
                m = tp.tile([128, KC * BL], f32, tag="m")
                nc.vector.scalar_tensor_tensor(m[:], tf, 1.0, cst[rd][:], op0=add, op1=mult)
                nc.vector.scalar_tensor_tensor(cst[wr][:], m[:], 0.5, v[:], op0=mult, op1=add)
                tcell = tp.tile([128, KC * BL], f32, tag="tcell")
                nc.scalar.activation(tcell[:], cst[wr][:], Tanh, scale=0.5)
                nc.vector.scalar_tensor_tensor(hst[wr][:], to, 1.0, tcell[:], op0=add, op1=mult)
                nc.vector.tensor_copy(hbf[wr][:], hst[wr][:])
                if layer == 0:
                    # append h0'' (bf16) to history at col (t+1)*8
                    hv = hst[wr][:].rearrange("p (k b) -> p k b", k=KC)
                    nc.vector.tensor_copy(
                        hist3[:, :, ds(iv * (64 * BL) + (j + 1) * BL, BL)], hv)
                else:
                    nc.sync.dma_start(
                        out=outb[ds(iv * (64 * 128) + j * 128, 128), :],
                        in_=hst[wr][:])

            # ---------------- Layer 0 ----------------
            def dma_xb(xbt, col_expr):
                nc.sync.dma_start(out=xbt[:].rearrange("p (k c) -> p k c", k=KC),
                                  in_=xt[:, :, col_expr])

            # prologue: block 0 -> zA, prefetch block 1
            dma_xb(xb[0], slice(0, 256))
            proj("ih0", lambda kc: xb[0][:, kc * 256: kc * 256 + 256], zt[0], 0)
            dma_xb(xb[1], slice(256, 512))

            with tc.For_i(0, NBLK // 2, 1) as i:
                proj("ih0", lambda kc: xb[1][:, kc * 256: kc * 256 + 256], zt[1], 0)
                for j in range(TBLK):
                    step(0, i, j, zt[0])
                dma_xb(xb[0], ds(i * 512 + 512, 256))
                proj("ih0", lambda kc: xb[0][:, kc * 256: kc * 256 + 256], zt[0], 0)
                for j in range(TBLK, 2 * TBLK):
                    step(0, i, j, zt[1])
                dma_xb(xb[1], ds(i * 512 + 768, 256))

            # ---------------- Layer 1 ----------------
            # prologue: Z1 for block 0
            proj("ih1", lambda kc: hist[:, kc * HS + 8: kc * HS + 8 + 256],
                 zt[0], MC)

            with tc.For_i(0, NBLK // 2, 1) as i:
                proj("ih1",
                     lambda kc: hist[:, ds(i * 512 + kc * HS + 256 + 8, 256)],
                     zt[1], MC)
                for j in range(TBLK):
                    step(1, i, j, zt[0])
                proj("ih1",
                     lambda kc: hist[:, ds(i * 512 + kc * HS + 512 + 8, 256)],
                     zt[0], MC)
                for j in range(TBLK, 2 * TBLK):
                    step(1, i, j, zt[1])

            # ---------------- epilogue: final states ----------------
            for row, t_ in enumerate((h0[0], c0[0], h1[0], c1[0])):
                nc.sync.dma_start(out=fin[row * 128: (row + 1) * 128, :], in_=t_[:])

    nc.compile()
    return nc


def _get_nc():
    if "nc" not in _NC_CACHE:
        _NC_CACHE["nc"] = _build_nc()
    return _NC_CACHE["nc"]


def _prep_weights(W_ih0, b_ih0, W_hh0, b_hh0, W_ih1, b_ih1, W_hh1, b_hh1):
    """Host-side weight adjustment + layout. Returns dict of shared arrays."""
    def adj(W, gscale2, half):
        Wa = np.asarray(W, np.float32).copy()
        Wa[3 * H:] *= 2.0          # tanh-trick: g rows doubled
        if half:
            Wa *= 0.5              # consumes h'' = 2h
        return Wa

    def lay(Wa):
        # lhsT tile [128, kc*GD + m]: tile[p, kc*GD+m] = Wa.T[kc*128+p, m]
        WT = Wa.T.astype(np.float32)                      # [din, GD]
        t = WT.reshape(KC, 128, GD).transpose(1, 0, 2).reshape(128, KC * GD)
        return np.ascontiguousarray(t.astype(BF16))

    def bias_col(b_ih, b_hh):
        ba = (np.asarray(b_ih, np.float32) + np.asarray(b_hh, np.float32)).copy()
        ba[3 * H:] *= 2.0
        # [128, MC]: col mc, partition p -> bias[mc*128+p]
        return ba.reshape(MC, 128).T.copy()

    out = {
        "w_ih0": lay(adj(W_ih0, True, False)),
        "w_hh0": lay(adj(W_hh0, True, True)),
        "w_ih1": lay(adj(W_ih1, True, True)),
        "w_hh1": lay(adj(W_hh1, True, True)),
    }
    b = np.zeros((128, 2 * MC), np.float32)
    b[:, :MC] = bias_col(b_ih0, b_hh0)
    b[:, MC:] = bias_col(b_ih1, b_hh1)
    out["biases"] = b
    return out


def _prep_x_core(x, core):
    xs = np.asarray(x[:, core * BL:(core + 1) * BL, :], np.float32)  # [T, BL, IN]
    # xt2[p, kc, t*BL+b] = x[t, b, kc*128+p]
    xt2 = xs.transpose(2, 0, 1).reshape(IN, T * BL)
    xt2 = xt2.reshape(KC, 128, T * BL).transpose(1, 0, 2)  # [128, KC, T*BL]
    xp = np.zeros((128, KC, XS), np.float32)
    xp[:, :, : T * BL] = xt2
    return np.ascontiguousarray(xp.astype(BF16))


def kernel(x, W_ih0, b_ih0, W_hh0, b_hh0, W_ih1, b_ih1, W_hh1, b_hh1):
    from concourse.bass_utils import run_bass_kernel_spmd

    nc = _get_nc()
    shared = _prep_weights(W_ih0, b_ih0, W_hh0, b_hh0,
                           W_ih1, b_ih1, W_hh1, b_hh1)
    in_maps = []
    for c in range(NCORES):
        m = dict(shared)
        m["xt"] = _prep_x_core(x, c)
        in_maps.append(m)

    r = run_bass_kernel_spmd(nc, in_maps, core_ids=list(range(NCORES)))

    outputs = np.empty((T, B, H), np.float32)
    hT = np.empty((L, B, H), np.float32)
    cT = np.empty((L, B, H), np.float32)
    for c in range(NCORES):
        ob = r.results[c]["outb"].reshape(T, 128, KC, BL)
        outputs[:, c * BL:(c + 1) * BL, :] = \
            0.5 * ob.transpose(0, 3, 2, 1).reshape(T, BL, H)
        fi = r.results[c]["fin"].reshape(4, 128, KC, BL)
        sl = fi.transpose(0, 3, 2, 1).reshape(4, BL, H)
        hT[0, c * BL:(c + 1) * BL] = 0.5 * sl[0]
        cT[0, c * BL:(c + 1) * BL] = 0.5 * sl[1]
        hT[1, c * BL:(c + 1) * BL] = 0.5 * sl[2]
        cT[1, c * BL:(c + 1) * BL] = 0.5 * sl[3]
    return outputs, hT, cT
